# revision 13
# baseline (speedup 1.0000x reference)
"""StyleGAN2-style modulated 3x3 conv (B=8, Ci=Co=512, H=W=32) on 8 TRN2 NeuronCores.

Sharding: data-parallel over batch, one sample per core (embarrassingly
parallel, no collectives).

Algorithm: Winograd F(2x2, 3x3). Per core:
  x ships pre-split by column parity (host): xt[j][par][r, q]
  mod:  parity planes P[par][j] = zero-padded(x * y_s)        (DVE TS)
  ct:   column-combine on parity planes (B^T d cols)          (DVE TT)
  v:    row-combine at stride-2 rows (B^T d B)                (DVE TT, 2x)
  M[xi,nu]   = sum_j U[xi,nu,j]^T @ V[j,xi,nu] (PE, fp32 PSUM, N=256)
  Y1[eta,nu] = xi-combine(M)                   (A^T M)        (DVE)
  Y[eta,mu]  = nu-combine(Y1)                  (A^T M A)      (DVE)
  ot[eta,mu,tile] = Y/rs + bias  (winograd layout; host descatters)
  rs = sqrt(sum_i ys2[i] w2[i,o] + eps)        demod (tiny PE matmuls)

Layout choices keep every hot DVE op on dense innermost runs so
tensor_tensor hits its 2x bf16 mode (the column-combine is inherently
one-element-offset and runs 1x, but on the smallest intermediate).
Output stays in winograd tile order so there is no strided scatter; the
host reshape is free.

Transformed weights U = G w G^T are computed host-side and streamed as
8 nu-paired 1MB slabs over three DMA queues in consumption order; demod
uses a separately shipped w2 = sum_k w_k^2.

Math note: the equal_lr scale s=(Ci*9)**-0.5 is folded out of both conv
and demod norm (eps compensated), so U/w2 come from the raw weights.
"""

import numpy as np
import ml_dtypes

import concourse.mybir as mybir
from concourse import bacc
from concourse.tile import TileContext
from concourse.bass_utils import run_bass_kernel_spmd

B = 8
CI = 512
CO = 512
H = W = 32
KK = 9
NCI = CI // 128
NCO = CO // 128
T = 16  # winograd tile grid (16x16 tiles of 2x2 outputs)
NPT = 256  # tiles per image = T*T
PR = 34  # padded rows
PW = 18  # parity-plane width (17 used + 1 pad for evenness)
EPS_EFF = 1e-8 * CI * KK

F32 = mybir.dt.float32
BF16 = mybir.dt.bfloat16
AF = mybir.ActivationFunctionType

# column-combine on parity planes. Plane 0 holds odd padded cols
# (p=2q+1, i.e. x even cols at idx 0..15, right-pad zero at idx 16);
# plane 1 holds even padded cols (p=2q: left-pad zero at idx 0, x odd
# cols at idx 1..16). Winograd col d_k = padded col 2tj+k:
#   d0 = plane1[tj], d1 = plane0[tj], d2 = plane1[tj+1], d3 = plane0[tj+1]
# nu0 = d0-d2, nu1 = d1+d2, nu2 = d2-d1, nu3 = d1-d3
# entries: (par_a, off_a, par_b, off_b, op)
CT_COMBINE = [
    (1, 0, 1, 1, "subtract"),
    (0, 0, 1, 1, "add"),
    (1, 1, 0, 0, "subtract"),
    (0, 0, 0, 1, "subtract"),
]
# row-combine (xi): rows a+2ti, b+2ti of ct
BT_ROWS = [(0, 2, "subtract"), (1, 2, "add"), (2, 1, "subtract"), (1, 3, "subtract")]


def build_nc():
    nc = bacc.Bacc("TRN2", target_bir_lowering=False, debug=False)

    # x: per j chunk [128, parity, 32, 16]
    x_ext = nc.declare_dram_parameter("x", [NCI, 128, 2, H, T], BF16, isOutput=False)
    yb_ext = nc.declare_dram_parameter("yb", [128, 2 * NCI], F32, isOutput=False)
    # transformed weights: [jo, nu, ci_p, xi, j, co_c]
    u_ext = nc.declare_dram_parameter(
        "u", [NCO, 4, 128, 4, NCI, 128], BF16, isOutput=False
    )
    # w2 = sum_k w_k^2: [ci_p, j, jo, co_c]
    w2_ext = nc.declare_dram_parameter("w2", [128, NCI, NCO, 128], BF16, isOutput=False)
    # out in winograd layout [jo, co_c, eta, mu, ti*16+tj], bf16 (host widens)
    out_ext = nc.declare_dram_parameter(
        "out", [NCO, 128, 2, 2, NPT], BF16, isOutput=True
    )

    with TileContext(nc) as tc:
        with (
            tc.tile_pool(name="singles", bufs=1) as singles,
            tc.tile_pool(name="us", bufs=12) as us,
            tc.tile_pool(name="big", bufs=1) as big,
            tc.tile_pool(name="xin", bufs=1) as xin,
            tc.tile_pool(name="y1s", bufs=1) as y1s,
            tc.tile_pool(name="tmps", bufs=2) as tmps,
            tc.tile_pool(name="outs", bufs=2) as outs,
            tc.tile_pool(name="cps", bufs=3, space="PSUM") as cps,
            tc.tile_pool(name="dps", bufs=1, space="PSUM") as dps,
            tc.tile_pool(name="wps", bufs=1, space="PSUM") as wps,
        ):
            xt_sb = [
                xin.tile([128, 2, H, T], BF16, tag=f"x{j}", name=f"xt{j}")
                for j in range(NCI)
            ]
            yb_sb = singles.tile([128, 2 * NCI], F32)
            w2_sb = singles.tile([128, NCI, NCO, 128], BF16)

            # parity planes + transform intermediates
            # pl[par][j]: [34 rows, 18]; ct[j, nu]: [34 rows, 16];
            # v[j, xi, nu]: [256]
            pl = big.tile([128, 2, NCI, PR, PW], BF16)
            ct_sb = big.tile([128, NCI, 4, PR, T], BF16)
            v_sb = big.tile([128, NCI, 4, 4, NPT], BF16)

            # ---- padding memsets first so the gpsimd queue clears them
            # before anything else (mod depends on them) ----
            nc.gpsimd.memset(pl[:, :, :, 0, :], 0.0)
            nc.gpsimd.memset(pl[:, :, :, PR - 1, :], 0.0)
            nc.gpsimd.memset(pl[:, 0, :, 1 : PR - 1, T], 0.0)
            nc.gpsimd.memset(pl[:, 1, :, 1 : PR - 1, 0], 0.0)
            # pre-load the Q7 tensor_tensor library so the first real gpsimd
            # combine doesn't pay the ~6us IRAM load
            gwarm = singles.tile([128, 2], BF16)
            nc.gpsimd.memset(gwarm, 0.0)
            nc.gpsimd.tensor_tensor(
                out=gwarm[:, 0:1],
                in0=gwarm[:, 0:1],
                in1=gwarm[:, 1:2],
                op=mybir.AluOpType.add,
            )

            # ---- input DMAs, priority order per queue ----
            u_sb = {}  # (jo, nu) -> [128, 4xi, 4j, 128co]

            def udma(engine, jo, nu):
                t = us.tile([128, 4, NCI, 128], BF16, tag="u", name=f"u{jo}{nu}")
                engine.dma_start(out=t, in_=u_ext[jo, nu])
                u_sb[(jo, nu)] = t

            # Two DMA queues only (a third just splits the same SDMA
            # bandwidth): even-jo slabs on sync, odd-jo on scalar, strictly
            # in consumption order k = 4*nu + jo. x gates the transform so
            # it goes first; w2 is only needed by demod (after round 0).
            # single DMA queue: one HWDGE ring sustains ~344 GB/s alone vs
            # ~357 for two, and keeping it on the (otherwise idle) sync
            # engine frees the ACT engine of all trigger work
            DIAG = [
                (jo, d - jo)
                for d in range(7)
                for jo in range(min(d, 3), max(d - 3, 0) - 1, -1)
            ]
            nc.sync.dma_start(out=yb_sb, in_=yb_ext[:, :])
            for j in range(NCI):
                nc.sync.dma_start(out=xt_sb[j], in_=x_ext[j])
            for i, (jo, nu) in enumerate(DIAG):
                udma(nc.sync, jo, nu)
                if i == 3:
                    nc.sync.dma_start(out=w2_sb, in_=w2_ext[:, :])

            # ---- PE warm-up (HAM clock gate) ----
            warm_lhs = singles.tile([128, 1], BF16)
            nc.vector.memset(warm_lhs, 1.0)
            warm_rhs = singles.tile([128, 512], BF16)
            nc.vector.memset(warm_rhs, 0.5)
            warm_ps = wps.tile([1, 512], F32)
            N_WARM = 16
            for i in range(N_WARM):
                nc.tensor.matmul(
                    out=warm_ps,
                    lhsT=warm_lhs,
                    rhs=warm_rhs,
                    start=(i == 0),
                    stop=(i == N_WARM - 1),
                )

            eps_sb = singles.tile([128, 1], F32)
            nc.vector.memset(eps_sb, EPS_EFF)

            def mod(j):
                # plane0 (data at idx 0..15) from x even cols; plane1 at 1..16
                # (on ACT so the DVE can start the combines immediately)
                nc.scalar.activation(
                    out=pl[:, 0, j, 1 : H + 1, 0:T],
                    in_=xt_sb[j][:, 0],
                    func=AF.Identity,
                    scale=yb_sb[:, j : j + 1],
                )
                nc.scalar.activation(
                    out=pl[:, 1, j, 1 : H + 1, 1 : T + 1],
                    in_=xt_sb[j][:, 1],
                    func=AF.Identity,
                    scale=yb_sb[:, j : j + 1],
                )

            def ctop(nu, j=None, eng=None):
                pa, oa, pb, ob, op = CT_COMBINE[nu]
                jj = slice(None) if j is None else slice(j, j + 1)
                (eng or nc.vector).tensor_tensor(
                    out=ct_sb[:, jj, nu],
                    in0=pl[:, pa, jj, :, oa : oa + T],
                    in1=pl[:, pb, jj, :, ob : ob + T],
                    op=getattr(mybir.AluOpType, op),
                )

            def vop(xi, nu):
                a, b, op = BT_ROWS[xi]
                nc.vector.tensor_tensor(
                    out=v_sb[:, :, xi, nu],
                    in0=ct_sb[:, :, nu, a : min(a + 2 * T, PR) : 2, :],
                    in1=ct_sb[:, :, nu, b : min(b + 2 * T, PR) : 2, :],
                    op=getattr(mybir.AluOpType, op),
                )

            for j in range(NCI):
                mod(j)
                ctop(0, j)
            ys2_sb = singles.tile([128, NCI], BF16)
            nc.vector.tensor_mul(ys2_sb, yb_sb[:, 0:NCI], yb_sb[:, 0:NCI])
            for xi in range(4):
                vop(xi, 0)
            for nu in range(1, 4):
                ctop(nu)
                for xi in range(4):
                    vop(xi, nu)

            rs_sb = singles.tile([128, NCO], F32)

            # per-jo evacuated M (bf16): [128, 4nu, 4xi, 256]
            m_sb = [
                y1s.tile([128, 4, 4, NPT], BF16, tag=f"m_{jo}", name=f"m_{jo}")
                for jo in range(NCO)
            ]
            # per-jo Y1 (bf16): [128, 2eta, 4nu, 256]
            y1_sb = [
                y1s.tile([128, 2, 4, NPT], BF16, tag=f"y1_{jo}", name=f"y1_{jo}")
                for jo in range(NCO)
            ]
            # per-jo Y (bf16): [128, 2eta, 2mu, 256]
            yt_sb = [
                y1s.tile([128, 2, 2, NPT], BF16, tag=f"yt_{jo}", name=f"yt_{jo}")
                for jo in range(NCO)
            ]
            ot_sb = [
                outs.tile([128, 2, 2, NPT], BF16, tag=f"ot{jo % 2}", name=f"ot{jo}")
                for jo in range(NCO)
            ]

            def unit(jo, nu):
                # M[xi] for this (jo, nu): 16 matmuls N=256, xi-outer so each
                # xi's accumulation chain is contiguous; one ACT op evacuates
                # all 4 xi slices to SBUF bf16.
                ps = cps.tile([128, 4, NPT], F32, tag="ups")
                u = u_sb[(jo, nu)]
                for xi in range(4):
                    for j in range(NCI):
                        nc.tensor.matmul(
                            out=ps[:, xi],
                            lhsT=u[:, xi, j],
                            rhs=v_sb[:, j, xi, nu],
                            start=(j == 0),
                            stop=(j == NCI - 1),
                        )
                nc.scalar.activation(out=m_sb[jo][:, nu], in_=ps, func=AF.Copy)

            TT = nc.vector.tensor_tensor
            ADD = mybir.AluOpType.add
            SUB = mybir.AluOpType.subtract

            def stage1(jo, n0, n1):
                # Y1[0,nu] = M0+M1+M2 ; Y1[1,nu] = M1-M2-M3 over nu in [n0,n1)
                m = m_sb[jo]
                y1 = y1_sb[jo]
                t = tmps.tile([128, 4, NPT], BF16, tag="t1", name="t")
                TT(out=t[:, n0:n1], in0=m[:, n0:n1, 0], in1=m[:, n0:n1, 1], op=ADD)
                TT(out=y1[:, 0, n0:n1], in0=t[:, n0:n1], in1=m[:, n0:n1, 2], op=ADD)
                TT(out=t[:, n0:n1], in0=m[:, n0:n1, 1], in1=m[:, n0:n1, 2], op=SUB)
                TT(out=y1[:, 1, n0:n1], in0=t[:, n0:n1], in1=m[:, n0:n1, 3], op=SUB)

            def stage1_piece(jo, nu):
                # one (jo, nu) eta-combine on gpsimd (slow engine, early gate)
                m = m_sb[jo]
                y1 = y1_sb[jo]
                t = tmps.tile([128, NPT], BF16, tag="tg", name="tg")
                G = nc.gpsimd.tensor_tensor
                G(out=t, in0=m[:, nu, 0], in1=m[:, nu, 1], op=ADD)
                G(out=y1[:, 0, nu], in0=t, in1=m[:, nu, 2], op=ADD)
                G(out=t, in0=m[:, nu, 1], in1=m[:, nu, 2], op=SUB)
                G(out=y1[:, 1, nu], in0=t, in1=m[:, nu, 3], op=SUB)

            def stage2(jo, mu):
                # Y[:,mu0] = Y1n0+Y1n1+Y1n2 ; Y[:,mu1] = Y1n1-Y1n2-Y1n3
                y1 = y1_sb[jo]
                yt = yt_sb[jo]
                op = ADD if mu == 0 else SUB
                na, nb, ncol = (0, 1, 2) if mu == 0 else (1, 2, 3)
                t = tmps.tile([128, 2, NPT], BF16, tag="t2", name="t")
                TT(out=t, in0=y1[:, :, na], in1=y1[:, :, nb], op=op)
                TT(out=yt[:, :, mu], in0=t, in1=y1[:, :, ncol], op=op)

            def finish(jo, mu):
                # ot[:, :, mu] = yt[:, :, mu]*rs + bias (winograd layout, no
                # strided scatter), on ACT to keep DVE free for the combines
                nc.scalar.activation(
                    out=ot_sb[jo][:, :, mu],
                    in_=yt_sb[jo][:, :, mu],
                    func=AF.Identity,
                    bias=yb_sb[:, NCI + jo : NCI + jo + 1],
                    scale=rs_sb[:, jo : jo + 1],
                )

            # ---- PE stream: nu-outer rounds; per-jo combine work staggered:
            # nu0-2 parts after round 2, nu3 parts + output after each
            # round-3 unit ----
            # diagonal wavefront: jo0 finishes its four units by position 9
            # (not 15), so each jo's combine work unlocks mid-stream and the
            # ACT evac+finish load stays under the matmul cycle budget.
            # gpsimd handles the eta-combine for jo0/jo1 nu<=2 (cheap pieces,
            # early gates); DVE handles the rest.
            xs2_ps = dps.tile([128, NCO], F32)

            def demod():
                for jo in range(NCO):
                    for j in range(NCI):
                        nc.tensor.matmul(
                            out=xs2_ps[:, jo : jo + 1],
                            lhsT=w2_sb[:, j, jo],
                            rhs=ys2_sb[:, j : j + 1],
                            start=(j == 0),
                            stop=(j == NCI - 1),
                        )
                nc.scalar.activation(
                    out=rs_sb, in_=xs2_ps, func=AF.Sqrt, bias=eps_sb
                )
                nc.vector.reciprocal(out=rs_sb, in_=rs_sb)

            def jo_tail(jo):
                # everything left for this jo once its nu=3 unit is queued
                stage1(jo, 3, 4)
                stage2(jo, 0)
                finish(jo, 0)
                nc.sync.dma_start(
                    out=out_ext[jo][:, :, 0], in_=ot_sb[jo][:, :, 0]
                )
                stage2(jo, 1)
                finish(jo, 1)
                nc.sync.dma_start(
                    out=out_ext[jo][:, :, 1], in_=ot_sb[jo][:, :, 1]
                )

            for i, (jo, nu) in enumerate(DIAG):
                unit(jo, nu)
                if jo <= 1 and nu <= 2:
                    stage1_piece(jo, nu)
                elif jo >= 2 and nu == 2:
                    stage1(jo, 0, 3)
                if i == 3:
                    demod()
                if nu == 3:
                    jo_tail(jo)

            warm_sink = singles.tile([1, 1], F32)
            nc.vector.tensor_copy(out=warm_sink, in_=warm_ps[0:1, 0:1])
    nc.compile()
    return nc


_NC_CACHE = None


def _get_nc():
    global _NC_CACHE
    if _NC_CACHE is None:
        _NC_CACHE = build_nc()
    return _NC_CACHE


_G = np.array(
    [[1, 0, 0], [0.5, 0.5, 0.5], [0.5, -0.5, 0.5], [0, 0, 1]], np.float64
)


def _prep_inputs(x, y_s, weight, bias):
    w = weight.astype(np.float64)
    # U[xi, nu, co, ci] = G w G^T (input-independent weight transform)
    u = np.einsum("xa,nb,oiab->xnoi", _G, _G, w)
    # arrange to [jo, nu, ci_p, xi, j, co_c]
    u6 = u.reshape(4, 4, NCO, 128, NCI, 128).transpose(2, 1, 5, 0, 4, 3)
    u_arr = np.ascontiguousarray(u6).astype(ml_dtypes.bfloat16)
    w2 = (w**2).sum(axis=(2, 3))  # [co, ci]
    # [ci_p, j, jo, co_c]
    w2_arr = np.ascontiguousarray(
        w2.reshape(NCO, 128, NCI, 128).transpose(3, 2, 0, 1)
    ).astype(ml_dtypes.bfloat16)
    in_maps = []
    # x parity split: [j, ci_p, parity, r, q]; parity0 = even cols,
    # parity1 = odd cols
    xb = x.reshape(B, NCI, 128, H, T, 2)
    for b in range(B):
        yb = np.empty((128, 2 * NCI), np.float32)
        yb[:, :NCI] = y_s[b].reshape(NCI, 128).T
        yb[:, NCI:] = bias.reshape(NCO, 128).T
        xs = np.ascontiguousarray(xb[b].transpose(0, 1, 4, 2, 3)).astype(
            ml_dtypes.bfloat16
        )
        in_maps.append({"x": xs, "yb": yb, "u": u_arr, "w2": w2_arr})
    return in_maps


def _install_trace_support():
    """Dev-only: register the axon NTFF profiling hook + disable the
    remote artifact upload so trace=True works in this container."""
    import sys
    import types

    import concourse.bass_utils as bu

    bu.upload_artifacts = lambda tmpdir: "local://" + str(tmpdir)
    if "antenv.axon_hooks" in sys.modules:
        return
    try:
        from trn_agent_boot.trn_boot import _ntff_profile_via_ctypes

        hook = _ntff_profile_via_ctypes("/opt/axon/libaxon_pjrt.so")
    except Exception:
        return
    mod = types.ModuleType("antenv.axon_hooks")
    mod.get_axon_ntff_profile_hook = lambda: hook
    mod.set_axon_ntff_profile_hook = lambda h: None
    sys.modules["antenv.axon_hooks"] = mod


def run(x, y_s, weight, bias, trace=False, tmpdir=None):
    nc = _get_nc()
    if trace:
        _install_trace_support()
    in_maps = _prep_inputs(x, y_s, weight, bias)
    res = run_bass_kernel_spmd(
        nc, in_maps, core_ids=list(range(B)), trace=trace, tmpdir=tmpdir
    )
    # descatter winograd layout: [jo, co_c, eta, mu, ti, tj] -> [co, h, w]
    out = np.stack(
        [
            res.results[b]["out"]
            .reshape(NCO, 128, 2, 2, T, T)
            .transpose(0, 1, 4, 2, 5, 3)
            .reshape(CO, H, W)
            for b in range(B)
        ]
    ).astype(np.float32)
    return out, res


def kernel(x, y_s, weight, bias):
    out, _ = run(
        np.asarray(x, dtype=np.float32),
        np.asarray(y_s, dtype=np.float32),
        np.asarray(weight, dtype=np.float32),
        np.asarray(bias, dtype=np.float32),
    )
    return out
